# revision 4
# baseline (speedup 1.0000x reference)
"""Trainium2 Bass kernel for the 32-iteration 3x3 survival automaton.

Problem: x is a 4096x4096 binary fp32 grid. 32 iterations of:
    keep cell iff its 8-neighbor live count > 3  (zero 'SAME' padding)
Output: scalar sum(x) - sum(y_final).

Strategy (8 NeuronCores, SPMD, zero inter-core communication):
  - Row-shard: core c owns rows [512c, 512c+512). Each core loads its rows
    plus a 32-row halo on each side; after 32 iterations the halo is fully
    consumed and the owned rows are exact. Plus 1 guard row/col of zeros
    emulating the 'SAME' zero padding (dead cells stay dead, so guards
    self-maintain).
  - Per-core slab: 578 rows x 4098 cols, stored bf16 as 5 SBUF tiles of 128
    partitions (rows), stride 126 (2-row overlap between tiles).
  - Update rule algebra: with B[c] = y[c-1] + y[c+1] (horizontal pair-sum,
    VectorE free-dim shifted add) the survival test is
        y_new = step( Tri@B + (Tri + 16 I)@y  - 20.5 )
    where Tri is the 128x128 tridiagonal all-ones band (vertical 3-tap conv
    on TensorE) and the +16*center folds the "cell must be alive" condition
    into a single threshold. s = S9 + 16 y, with S9 the full 3x3 sum incl.
    center; alive cells give s >= 21 iff neighbors > 3, dead cells s <= 8.
  - Threshold: ScalarE Sigmoid(120*(s-20.5)) saturates to exactly 1.0 for
    alive (arg >= +60) and ~1e-26 for dead; VectorE is_gt handles the rest of
    the chunks exactly. Split tuned for engine balance.
  - Tile seams: after each iteration the 2-row tile overlaps go stale; 8 tiny
    SBUF->SBUF DMAs refresh them.
  - Final reduction: masked ones-vector matmuls accumulate column sums of the
    owned 512 rows into one PSUM bank; VectorE reduces to a scalar per core.
    Host sums the 8 partials and subtracts from sum(x).
"""

import sys

if '/opt/trn_rl_repo' not in sys.path:
    sys.path.insert(0, '/opt/trn_rl_repo')

from contextlib import ExitStack

import ml_dtypes
import numpy as np

import concourse.bass as bass
import concourse.tile as tile
from concourse import bacc, mybir
from concourse.bass_utils import run_bass_kernel_spmd

# ---------------------------------------------------------------- geometry
H = W = 4096
NCORES = 8
OWN = H // NCORES            # 512 rows owned per core
HALO = 32                    # rows of redundant compute per side
SLAB_R = OWN + 2 * HALO + 2  # 578 (incl. 1 guard row each side)
SLAB_C = W + 2               # 4098 (incl. 1 guard col each side)
NT = 5                       # SBUF row-tiles per slab
STRIDE = 126                 # tile row stride (2-row overlap)
OFF = [t * STRIDE for t in range(NT)]              # 0,126,252,378,504
RT = [min(128, SLAB_R - o) for o in OFF]           # 128,128,128,128,74
NCH = 8                      # 512-col PSUM chunks over the 4096 data cols
CHW = 512

# threshold split: chunks j < ACT_CHUNKS go to ScalarE (sigmoid), rest to
# VectorE (exact is_gt)
ACT_CHUNKS = 6
# B-pass (horizontal pair-sum) engine per tile: 'v' VectorE, 'g' GpSimd
B_ENGINE = ['v', 'v', 'v', 'g', 'g']

F32 = mybir.dt.float32
BF16 = mybir.dt.bfloat16


def _build(iters: int):
    nc = bacc.Bacc("TRN2", target_bir_lowering=False, debug=False)
    x_d = nc.dram_tensor("x", [SLAB_R, SLAB_C], F32, kind="ExternalInput").ap()
    tri_d = nc.dram_tensor("tri", [128, 128], BF16, kind="ExternalInput").ap()
    m16_d = nc.dram_tensor("m16", [128, 128], BF16, kind="ExternalInput").ap()
    rmask_d = nc.dram_tensor("rmask", [NT, 128], BF16, kind="ExternalInput").ap()
    out_d = nc.dram_tensor("ysum", [1, 1], F32, kind="ExternalOutput").ap()

    add = mybir.AluOpType.add

    with tile.TileContext(nc) as tc, ExitStack() as ctx:
        const_pool = ctx.enter_context(tc.tile_pool(name="const", bufs=1))
        ypool = ctx.enter_context(tc.tile_pool(name="y", bufs=1))
        bpool = ctx.enter_context(tc.tile_pool(name="b", bufs=1))
        stage_pool = ctx.enter_context(tc.tile_pool(name="stage", bufs=2))
        psum_pool = ctx.enter_context(tc.tile_pool(name="ps", bufs=7, space="PSUM"))
        spsum_pool = ctx.enter_context(tc.tile_pool(name="sps", bufs=1, space="PSUM"))

        tri_sb = const_pool.tile([128, 128], BF16, tag="tri")
        nc.sync.dma_start(tri_sb[:], tri_d[:])
        m16_sb = const_pool.tile([128, 128], BF16, tag="m16")
        nc.sync.dma_start(m16_sb[:], m16_d[:])
        rmask_sb = []
        for t in range(NT):
            rm = const_pool.tile([128, 1], BF16, tag=f"rmask{t}", name=f"rmask{t}")
            nc.sync.dma_start(rm[:], rmask_d[t:t + 1, :])
            rmask_sb.append(rm)
        bias_sb = const_pool.tile([128, 1], F32, tag="biasc", name="biasc")
        nc.gpsimd.memset(bias_sb[:], -2460.0)

        y_sb = [ypool.tile([RT[t], SLAB_C], BF16, tag=f"y{t}", name=f"y{t}") for t in range(NT)]
        b_sb = [bpool.tile([RT[t], W], BF16, tag=f"b{t}", name=f"b{t}") for t in range(NT)]

        # load + cast fp32 -> bf16
        for t in range(NT):
            stg = stage_pool.tile([128, SLAB_C], F32, tag="stage", name=f"stg{t}")
            nc.sync.dma_start(stg[0:RT[t], :], x_d[OFF[t]:OFF[t] + RT[t], :])
            nc.vector.tensor_copy(y_sb[t][:], stg[0:RT[t], :])

        for it in range(iters):
            for t in range(NT):
                eng = nc.vector if B_ENGINE[t] == 'v' else nc.gpsimd
                eng.tensor_tensor(
                    b_sb[t][:], y_sb[t][:, 0:W], y_sb[t][:, 2:W + 2], op=add)
            for t in range(NT):
                r = RT[t]
                psums = []
                for j in range(NCH):
                    ps = psum_pool.tile([r, CHW], F32, tag="ps", name=f"ps_{it}_{t}_{j}")
                    nc.tensor.matmul(
                        ps[:], tri_sb[0:r, 0:r],
                        b_sb[t][:, j * CHW:(j + 1) * CHW],
                        start=True, stop=False)
                    psums.append(ps)
                for j in range(NCH):
                    nc.tensor.matmul(
                        psums[j][:], m16_sb[0:r, 0:r],
                        y_sb[t][:, 1 + j * CHW:1 + (j + 1) * CHW],
                        start=False, stop=True)
                for j in range(NCH):
                    dst = y_sb[t][:, 1 + j * CHW:1 + (j + 1) * CHW]
                    if j < ACT_CHUNKS:
                        nc.scalar.activation(
                            dst, psums[j][:],
                            mybir.ActivationFunctionType.Sigmoid,
                            bias=bias_sb[0:r, 0:1], scale=120.0)
                    else:
                        nc.vector.tensor_scalar(
                            dst, psums[j][:], 20.5, None,
                            op0=mybir.AluOpType.is_gt)
            if it != iters - 1:
                for t in range(NT - 1):
                    # tile t's last row (stale) <- tile t+1 row 1 (fresh)
                    nc.sync.dma_start(
                        y_sb[t][RT[t] - 1:RT[t], :], y_sb[t + 1][1:2, :])
                    # tile t+1 row 0 (stale) <- tile t row 126 (fresh)
                    nc.sync.dma_start(
                        y_sb[t + 1][0:1, :], y_sb[t][STRIDE:STRIDE + 1, :])

        # masked column-sum of owned rows, accumulated in one PSUM bank
        sps = spsum_pool.tile([1, CHW], F32, tag="sum", name="sps")
        n_mm = NT * NCH
        k = 0
        for t in range(NT):
            for j in range(NCH):
                nc.tensor.matmul(
                    sps[:], rmask_sb[t][0:RT[t], 0:1],
                    y_sb[t][:, 1 + j * CHW:1 + (j + 1) * CHW],
                    start=(k == 0), stop=(k == n_mm - 1))
                k += 1
        ssb = const_pool.tile([1, 1], F32, tag="ssum", name="ssb")
        nc.vector.tensor_reduce(
            ssb[:], sps[:], axis=mybir.AxisListType.X, op=add)
        nc.sync.dma_start(out_d[:], ssb[:])

    nc.compile()
    return nc


def _consts():
    i = np.arange(128)
    tri = (np.abs(i[:, None] - i[None, :]) <= 1).astype(np.float32)
    m16 = tri + 16.0 * np.eye(128, dtype=np.float32)
    # valid-row masks for the final sum: slab rows [33, 545) are the owned
    # 512 rows; each row is summed from the tile where it is seam-valid
    # (interior partitions [1, 127) after the last iteration).
    rmask = np.zeros((NT, 128), np.float32)
    lo, hi = HALO + 1, HALO + 1 + OWN      # [33, 545)
    bounds = [(33, 127), (1, 127), (1, 127), (1, 127), (1, 41)]
    for t, (a, b) in enumerate(bounds):
        rmask[t, a:b] = 1.0
        assert OFF[t] + a >= lo or t == 0
    assert sum(b - a for a, b in bounds) == OWN
    bf = ml_dtypes.bfloat16
    return tri.astype(bf), m16.astype(bf), rmask.astype(bf)


def _slabs(x: np.ndarray):
    g = np.zeros((H + 2 * HALO + 2, SLAB_C), np.float32)
    g[HALO + 1:HALO + 1 + H, 1:1 + W] = x
    return [np.ascontiguousarray(g[c * OWN:c * OWN + SLAB_R])
            for c in range(NCORES)]


_CACHE = {}


def _get_nc(iters: int):
    if iters not in _CACHE:
        _CACHE[iters] = _build(iters)
    return _CACHE[iters]


def kernel(x: np.ndarray, convs) -> np.ndarray:
    iters = int(convs)
    x = np.asarray(x, np.float32)
    assert x.shape == (H, W)
    nc = _get_nc(iters)
    tri, m16, rmask = _consts()
    in_maps = [{"x": s, "tri": tri, "m16": m16, "rmask": rmask}
               for s in _slabs(x)]
    res = run_bass_kernel_spmd(nc, in_maps, core_ids=list(range(NCORES)))
    y_sum = sum(float(res.results[c]["ysum"][0, 0]) for c in range(NCORES))
    x_sum = float(x.astype(np.float64).sum())
    return np.float32(x_sum - y_sum)


if __name__ == "__main__":
    # quick self-check against scipy on the full problem
    rng = np.random.default_rng(0)
    x = np.round(rng.random((H, W))).astype(np.float32)
    got = kernel(x, 32)
    from scipy import signal
    K = np.array([[1, 1, 1], [1, 0, 1], [1, 1, 1]], np.float32)
    y = x.copy()
    for _ in range(32):
        s = signal.convolve2d(y, K, mode='same')
        y = np.where(s > 3.0, y, 0).astype(np.float32)
    want = x.sum(dtype=np.float64) - y.sum(dtype=np.float64)
    print(f"got {got}, want {want}, rel {abs(got - want) / abs(want):.3e}")
